# revision 16
# baseline (speedup 1.0000x reference)
"""Batched square-root Kalman filter on 8 Trainium2 NeuronCores.

Strategy
--------
The covariance/gain recursion is data-independent, tiny (8x8), and converges to
steady state -- computed on HOST in float64.  The batched state recursion

    x_t = x_{t-1} @ A_t + y_t @ B_t,   A_t = F^T (I - H^T K_t^T),  B_t = K_t^T

is a linear time-varying recurrence whose impulse response decays geometrically
(closed-loop filter), so x_t depends only on the last W observations:

    x_t = sum_{s=t-W+1..t} y_s @ C_{s,t} + x0 @ M_t,   C_{s,t} = B_s A_{s+1}..A_t

With a pseudo-step s=0 carrying x0 (C_{0,t} = M_t) this is ONE banded
block-triangular matmul, embarrassingly parallel over time and batch.  The
operator is Toeplitz after Riccati convergence (~75 steps), so only ~30 unique
128x128 weight tiles exist.  Device work per core = 305 accumulating
[128x128]@[128x256] fp32r matmuls + PSUM->SBUF copies + streaming DMA.

Batch dim B=2048 is sharded 8 ways (256 seqs/core); weights are replicated.
Host packs Y into time-major partition-major layout so every DMA is contiguous.
"""

import os

import numpy as np

T, NS, ND = 1000, 8, 8
B, NCORES = 2048, 8
BL = B // NCORES                 # 256 sequences per core
NK = 63                          # number of 128-row tiles
NROW = NK * 128                  # 8064 rows; row r = 8*s + i, s=0 is x0 slot
SS_TOL = 1e-7                    # Riccati steady-state snap tolerance

# Precision mode.  bf16 I/O halves DMA traffic and its quantization noise
# (~3e-3 scale-relative) dominates band truncation, so a 4-tile band is free.
# KF_F32=1 switches to full fp32 I/O with a 5-tile band (~3.5e-4 rel).
F32_MODE = bool(os.environ.get("KF_F32"))
BAND_TILES = 5 if F32_MODE else 4   # 128-row k-tiles per output m-tile
W = 16 * BAND_TILES                 # max impulse-response window kept

LAST_RESULTS = {}                # test harness introspection (exec time etc.)


# ---------------------------------------------------------------- host math --
def _kf_gains(F, G, H, Q, R, P0):
    """Square-root KF covariance recursion (f64), faithful to the reference."""
    F, G, H, Q, R, P0 = [np.asarray(a, np.float64) for a in (F, G, H, Q, R, P0)]
    n, d = F.shape[0], H.shape[0]
    Qs = np.linalg.cholesky(Q)
    Rs = np.linalg.cholesky(R)
    P = P0.copy()
    As = np.empty((T + 1, n, n))
    Bs = np.empty((T + 1, d, n))
    Ps = np.empty((T, n, n))
    for t in range(1, T + 1):
        S_pos = np.linalg.cholesky(P)
        A_state = np.concatenate([F @ S_pos, G @ Qs], axis=1)
        Ra = np.linalg.qr(A_state.T)[1]
        S_neg = Ra.T[:, :n]
        top = np.concatenate([Rs, H @ S_neg], axis=1)
        bot = np.concatenate([np.zeros((n, d)), S_neg], axis=1)
        L = np.linalg.qr(np.concatenate([top, bot], axis=0).T)[1].T
        K = np.linalg.solve(L[:d, :d].T, L[d:, :d].T).T
        P = L[d:, d:] @ L[d:, d:].T
        As[t] = F.T @ (np.eye(n) - H.T @ K.T)
        Bs[t] = K.T
        Ps[t - 1] = P
    return As, Bs, Ps


def _build_operator(F, G, H, Q, R, P0):
    """Banded operator tiles.  Returns (uniq [U,128,128] f32, band, Ps f32)."""
    As, Bs, Ps = _kf_gains(F, G, H, Q, R, P0)
    n = NS
    A_ss, B_ss = As[T], Bs[T]
    dev = np.maximum(np.abs(As[1:] - A_ss).max(axis=(1, 2)),
                     np.abs(Bs[1:] - B_ss).max(axis=(1, 2)))
    t_ss = T
    for t in range(1, T + 1):
        if dev[t - 1:].max() < SS_TOL:
            t_ss = t
            break
    As[t_ss:] = A_ss          # snap -> post-convergence tiles dedupe exactly
    Bs[t_ss:] = B_ss

    S = NROW // 8
    tiles = {}
    win = {0: np.eye(n)}

    def put(s, t, C):
        kt, mt = (8 * s) // 128, (8 * t) // 128
        if kt < mt - (BAND_TILES - 1):      # outside allocated band tiles
            return
        blk = tiles.setdefault((kt, mt), np.zeros((128, 128)))
        blk[8 * s - 128 * kt:8 * s - 128 * kt + 8,
            8 * t - 128 * mt:8 * t - 128 * mt + 8] = C

    for t in range(S):
        if t > 0:
            A_t = As[t] if t <= T else A_ss
            for s in list(win):
                win[s] = win[s] @ A_t
                if t - s >= W:
                    del win[s]
            win[t] = Bs[t] if t <= T else B_ss
        for s, C in win.items():
            put(s, t, C)

    uniq, index, order = {}, {}, []
    for key in sorted(tiles):
        a = tiles[key].astype(np.float32)
        h = a.tobytes()
        if h not in uniq:
            uniq[h] = len(order)
            order.append(a)
        index[key] = uniq[h]
    band = [[(kt, index[(kt, mt)])
             for kt in range(max(0, mt - (BAND_TILES - 1)), mt + 1)
             if (kt, mt) in index]
            for mt in range(NK)]
    return np.stack(order), band, Ps.astype(np.float32)


# ------------------------------------------------------------- data packing --
def _pack_y(Y, x0, dtype):
    """[B,T,8]+[B,8] -> per-core [128, NK*BL], time-major partition-major."""
    Yrow = np.zeros((NROW, B), np.float32)
    Yrow[0:8] = np.asarray(x0, np.float32).T
    Yrow[8:8 + 8 * T] = np.asarray(Y, np.float32).reshape(B, 8 * T).T
    Yrow = Yrow.astype(dtype)
    out = []
    for c in range(NCORES):
        blk = Yrow[:, c * BL:(c + 1) * BL]                      # [8064, 256]
        out.append(np.ascontiguousarray(
            blk.reshape(NK, 128, BL).transpose(1, 0, 2)).reshape(128, NK * BL))
    return out


def _unpack_out(res):
    """per-core [128, NK*BL] -> traj [B,T,8] f32."""
    traj = np.empty((B, T, 8), np.float32)
    for c, o in enumerate(res):
        rows = o.astype(np.float32).reshape(128, NK, BL) \
                .transpose(1, 0, 2).reshape(NROW, BL)
        traj[c * BL:(c + 1) * BL] = rows[8:8 + 8 * T].T.reshape(BL, T, 8)
    return traj


# -------------------------------------------------------------- bass kernel --
def _build_bass(n_uniq, band, ych=8, gm=8, io_bf16=False, out_bf16=False,
                ob_bufs=3, ps_bufs=6):
    import concourse.bacc as bacc
    import concourse.mybir as mybir
    from concourse.tile import TileContext

    f32 = mybir.dt.float32
    f32r = mybir.dt.float32r
    bf16 = mybir.dt.bfloat16
    ydt = bf16 if io_bf16 else f32r
    odt = bf16 if out_bf16 else f32
    nc = bacc.Bacc("TRN2", target_bir_lowering=False)
    ydev = nc.dram_tensor("yin", [128, NK * BL], ydt, kind="ExternalInput")
    wdev = nc.dram_tensor("win", [128, n_uniq * 128], ydt, kind="ExternalInput")
    odev = nc.dram_tensor("out", [128, NK * BL], odt, kind="ExternalOutput")

    # Engine instructions carry limited semaphore waits; Bacc.compile()'s
    # generate_event_semaphores legalizes the rest.  The warmup matmul and
    # ascending band order keep the hot loop's waits cheap regardless.
    with TileContext(nc) as tc:
        with tc.tile_pool(name="resident", bufs=1) as rpool, \
             tc.tile_pool(name="ob", bufs=ob_bufs) as opool, \
             tc.tile_pool(name="ps", bufs=ps_bufs, space="PSUM") as pspool:
            wbuf = rpool.tile([128, n_uniq * 128], ydt)
            ybuf = rpool.tile([128, NK * BL], ydt)
            nc.sync.dma_start(out=wbuf[:], in_=wdev[:])
            for c0 in range(0, NK, ych):
                c1 = min(NK, c0 + ych)
                nc.sync.dma_start(out=ybuf[:, c0 * BL:c1 * BL],
                                  in_=ydev[:, c0 * BL:c1 * BL])
            warm = pspool.tile([128, 8], f32, tag="warm", bufs=1)
            nc.tensor.matmul(warm[:, :8], wbuf[:, :128], wbuf[:, :8],
                             start=True, stop=True)
            for g0 in range(0, NK, gm):
                g1 = min(NK, g0 + gm)
                obuf = opool.tile([128, gm * BL], odt, tag="obuf")
                for mt in range(g0, g1):
                    ps = pspool.tile([128, BL], f32, tag="ps")
                    lst = band[mt]
                    for j, (kt, u) in enumerate(lst):
                        nc.tensor.matmul(ps[:],
                                         wbuf[:, u * 128:(u + 1) * 128],
                                         ybuf[:, kt * BL:(kt + 1) * BL],
                                         start=(j == 0), stop=(j == len(lst) - 1))
                    nc.vector.tensor_copy(
                        out=obuf[:, (mt - g0) * BL:(mt - g0 + 1) * BL], in_=ps[:])
                nc.sync.dma_start(out=odev[:, g0 * BL:g1 * BL],
                                  in_=obuf[:, :(g1 - g0) * BL])
    nc.compile()
    return nc


# -------------------------------------------------------------- entry point --
def kernel(Y, F, G, H, Q, R, x0, P0):
    import ml_dtypes

    from concourse import bass_utils

    io_dtype = np.float32 if F32_MODE else ml_dtypes.bfloat16
    uniq, band, Ps = _build_operator(F, G, H, Q, R, P0)
    n_uniq = len(uniq)
    wpack = np.ascontiguousarray(
        uniq.transpose(1, 0, 2)).reshape(128, n_uniq * 128).astype(io_dtype)

    nc = _build_bass(n_uniq, band, ych=4, gm=4, ob_bufs=6, ps_bufs=7,
                     io_bf16=not F32_MODE, out_bf16=not F32_MODE)
    ycores = _pack_y(Y, x0, io_dtype)
    in_maps = [{"yin": yc, "win": wpack} for yc in ycores]

    trace = bool(os.environ.get("KF_TRACE"))
    res = bass_utils.run_bass_kernel_spmd(
        nc, in_maps, core_ids=list(range(NCORES)), trace=trace)
    LAST_RESULTS["bass"] = res
    LAST_RESULTS["nc"] = nc
    LAST_RESULTS["in_maps"] = in_maps

    traj = _unpack_out([r["out"] for r in res.results])
    return traj, Ps


# revision 24
# speedup vs baseline: 1.0147x; 1.0147x over previous
"""Batched square-root Kalman filter on 8 Trainium2 NeuronCores.

Strategy
--------
The covariance/gain recursion is data-independent, tiny (8x8), and converges to
steady state -- computed on HOST in float64.  The batched state recursion

    x_t = x_{t-1} @ A_t + y_t @ B_t,   A_t = F^T (I - H^T K_t^T),  B_t = K_t^T

is a linear time-varying recurrence whose impulse response decays geometrically
(closed-loop filter), so x_t depends only on the last W observations:

    x_t = sum_{s=t-W+1..t} y_s @ C_{s,t} + x0 @ M_t,   C_{s,t} = B_s A_{s+1}..A_t

With a pseudo-step s=0 carrying x0 (C_{0,t} = M_t) this is ONE banded
block-triangular matmul, embarrassingly parallel over time and batch.  The
operator is Toeplitz after Riccati convergence (~75 steps), so only ~30 unique
128x128 weight tiles exist.  Device work per core = 305 accumulating
[128x128]@[128x256] fp32r matmuls + PSUM->SBUF copies + streaming DMA.

Batch dim B=2048 is sharded 8 ways (256 seqs/core); weights are replicated.
Host packs Y into time-major partition-major layout so every DMA is contiguous.
"""

import os

import numpy as np

T, NS, ND = 1000, 8, 8
B, NCORES = 2048, 8
BL = B // NCORES                 # 256 sequences per core
NK = 63                          # number of 128-row tiles
NROW = NK * 128                  # 8064 rows; row r = 8*s + i, s=0 is x0 slot
SS_TOL = 1e-7                    # Riccati steady-state snap tolerance

# Precision mode.  bf16 I/O halves DMA traffic and its quantization noise
# (~3e-3 scale-relative) dominates band truncation, so a 4-tile band is free.
# KF_F32=1 switches to full fp32 I/O with a 5-tile band (~3.5e-4 rel).
F32_MODE = bool(os.environ.get("KF_F32"))
BAND_TILES = 5 if F32_MODE else 4   # 128-row k-tiles per output m-tile
W = 16 * BAND_TILES                 # max impulse-response window kept

LAST_RESULTS = {}                # test harness introspection (exec time etc.)


# ---------------------------------------------------------------- host math --
def _kf_gains(F, G, H, Q, R, P0):
    """Square-root KF covariance recursion (f64), faithful to the reference."""
    F, G, H, Q, R, P0 = [np.asarray(a, np.float64) for a in (F, G, H, Q, R, P0)]
    n, d = F.shape[0], H.shape[0]
    Qs = np.linalg.cholesky(Q)
    Rs = np.linalg.cholesky(R)
    P = P0.copy()
    As = np.empty((T + 1, n, n))
    Bs = np.empty((T + 1, d, n))
    Ps = np.empty((T, n, n))
    for t in range(1, T + 1):
        S_pos = np.linalg.cholesky(P)
        A_state = np.concatenate([F @ S_pos, G @ Qs], axis=1)
        Ra = np.linalg.qr(A_state.T)[1]
        S_neg = Ra.T[:, :n]
        top = np.concatenate([Rs, H @ S_neg], axis=1)
        bot = np.concatenate([np.zeros((n, d)), S_neg], axis=1)
        L = np.linalg.qr(np.concatenate([top, bot], axis=0).T)[1].T
        K = np.linalg.solve(L[:d, :d].T, L[d:, :d].T).T
        P = L[d:, d:] @ L[d:, d:].T
        As[t] = F.T @ (np.eye(n) - H.T @ K.T)
        Bs[t] = K.T
        Ps[t - 1] = P
    return As, Bs, Ps


def _build_operator(F, G, H, Q, R, P0):
    """Banded operator tiles.  Returns (uniq [U,128,128] f32, band, Ps f32)."""
    As, Bs, Ps = _kf_gains(F, G, H, Q, R, P0)
    n = NS
    A_ss, B_ss = As[T], Bs[T]
    dev = np.maximum(np.abs(As[1:] - A_ss).max(axis=(1, 2)),
                     np.abs(Bs[1:] - B_ss).max(axis=(1, 2)))
    t_ss = T
    for t in range(1, T + 1):
        if dev[t - 1:].max() < SS_TOL:
            t_ss = t
            break
    As[t_ss:] = A_ss          # snap -> post-convergence tiles dedupe exactly
    Bs[t_ss:] = B_ss

    S = NROW // 8
    tiles = {}
    win = {0: np.eye(n)}

    def put(s, t, C):
        kt, mt = (8 * s) // 128, (8 * t) // 128
        if kt < mt - (BAND_TILES - 1):      # outside allocated band tiles
            return
        blk = tiles.setdefault((kt, mt), np.zeros((128, 128)))
        blk[8 * s - 128 * kt:8 * s - 128 * kt + 8,
            8 * t - 128 * mt:8 * t - 128 * mt + 8] = C

    for t in range(S):
        if t > 0:
            A_t = As[t] if t <= T else A_ss
            for s in list(win):
                win[s] = win[s] @ A_t
                if t - s >= W:
                    del win[s]
            win[t] = Bs[t] if t <= T else B_ss
        for s, C in win.items():
            put(s, t, C)

    # dedupe, numbering tiles in first-use (execution) order so the early
    # slice of the packed weight tensor is exactly what the first m-tiles need
    uniq, index, order = {}, {}, []
    for mt in range(NK):
        for kt in range(max(0, mt - (BAND_TILES - 1)), mt + 1):
            if (kt, mt) not in tiles:
                continue
            a = tiles[(kt, mt)].astype(np.float32)
            h = a.tobytes()
            if h not in uniq:
                uniq[h] = len(order)
                order.append(a)
            index[(kt, mt)] = uniq[h]
    band = [[(kt, index[(kt, mt)])
             for kt in range(max(0, mt - (BAND_TILES - 1)), mt + 1)
             if (kt, mt) in index]
            for mt in range(NK)]
    return np.stack(order), band, Ps.astype(np.float32)


# ------------------------------------------------------------- data packing --
def _pack_y(Y, x0, dtype):
    """[B,T,8]+[B,8] -> per-core [128, NK*BL], time-major partition-major."""
    Yrow = np.zeros((NROW, B), np.float32)
    Yrow[0:8] = np.asarray(x0, np.float32).T
    Yrow[8:8 + 8 * T] = np.asarray(Y, np.float32).reshape(B, 8 * T).T
    Yrow = Yrow.astype(dtype)
    out = []
    for c in range(NCORES):
        blk = Yrow[:, c * BL:(c + 1) * BL]                      # [8064, 256]
        out.append(np.ascontiguousarray(
            blk.reshape(NK, 128, BL).transpose(1, 0, 2)).reshape(128, NK * BL))
    return out


def _unpack_out(res):
    """per-core [128, NK*BL] -> traj [B,T,8] f32."""
    traj = np.empty((B, T, 8), np.float32)
    for c, o in enumerate(res):
        rows = o.astype(np.float32).reshape(128, NK, BL) \
                .transpose(1, 0, 2).reshape(NROW, BL)
        traj[c * BL:(c + 1) * BL] = rows[8:8 + 8 * T].T.reshape(BL, T, 8)
    return traj


# -------------------------------------------------------------- bass kernel --
def _build_bass(n_uniq, band, ych=8, gm=8, io_bf16=False, out_bf16=False,
                ob_bufs=3, ps_bufs=6, store_ring="sync", copy_split=False):
    import concourse.bacc as bacc
    import concourse.mybir as mybir
    from concourse.tile import TileContext

    f32 = mybir.dt.float32
    f32r = mybir.dt.float32r
    bf16 = mybir.dt.bfloat16
    ydt = bf16 if io_bf16 else f32r
    odt = bf16 if out_bf16 else f32
    nc = bacc.Bacc("TRN2", target_bir_lowering=False)
    ydev = nc.dram_tensor("yin", [128, NK * BL], ydt, kind="ExternalInput")
    wdev = nc.dram_tensor("win", [128, n_uniq * 128], ydt, kind="ExternalInput")
    odev = nc.dram_tensor("out", [128, NK * BL], odt, kind="ExternalOutput")

    # Engine instructions carry limited semaphore waits; Bacc.compile()'s
    # generate_event_semaphores legalizes the rest.  The warmup matmul and
    # ascending band order keep the hot loop's waits cheap regardless.
    with TileContext(nc) as tc:
        with tc.tile_pool(name="resident", bufs=1) as rpool, \
             tc.tile_pool(name="ob", bufs=ob_bufs) as opool, \
             tc.tile_pool(name="ps", bufs=ps_bufs, space="PSUM") as pspool:
            wbuf = rpool.tile([128, n_uniq * 128], ydt)
            ybuf = rpool.tile([128, NK * BL], ydt)
            if isinstance(ych, int):
                chunks = [ych] * ((NK + ych - 1) // ych)
            else:
                chunks = list(ych)
            c0 = 0
            first = True
            wsplit = min(max(len(band[0]) + len(band[1]) + len(band[2]), 1),
                         n_uniq)
            for step in chunks:
                c1 = min(NK, c0 + step)
                nc.sync.dma_start(out=ybuf[:, c0 * BL:c1 * BL],
                                  in_=ydev[:, c0 * BL:c1 * BL])
                if first:
                    # weight tiles for the early m-tiles right after y chunk 0
                    # (tiles are packed in first-use order); the bulk of the
                    # (Toeplitz) weights can land later.
                    nc.sync.dma_start(out=wbuf[:, :wsplit * 128],
                                      in_=wdev[:, :wsplit * 128])
                    if wsplit < n_uniq:
                        nc.sync.dma_start(out=wbuf[:, wsplit * 128:],
                                          in_=wdev[:, wsplit * 128:])
                    first = False
                c0 = c1
                if c0 >= NK:
                    break
            warm = pspool.tile([128, 8], f32, tag="warm", bufs=1)
            nc.tensor.matmul(warm[:, :8], wbuf[:, :128], wbuf[:, :8],
                             start=True, stop=True)
            store_eng = nc.scalar if store_ring == "scalar" else nc.sync
            if isinstance(gm, int):
                groups = []
                g0 = 0
                while g0 < NK:
                    groups.append((g0, min(NK, g0 + gm)))
                    g0 += gm
            else:
                groups = list(gm)
            gmax = max(g1 - g0 for g0, g1 in groups)
            for gi, (g0, g1) in enumerate(groups):
                obuf = opool.tile([128, gmax * BL], odt, tag="obuf")
                tail = gi >= len(groups) - 2
                for mt in range(g0, g1):
                    ps = pspool.tile([128, BL], f32, tag="ps")
                    lst = band[mt]
                    for j, (kt, u) in enumerate(lst):
                        nc.tensor.matmul(ps[:],
                                         wbuf[:, u * 128:(u + 1) * 128],
                                         ybuf[:, kt * BL:(kt + 1) * BL],
                                         start=(j == 0), stop=(j == len(lst) - 1))
                    dst = obuf[:, (mt - g0) * BL:(mt - g0 + 1) * BL]
                    if (copy_split or tail) and mt % 2 == 1:
                        nc.scalar.copy(out=dst, in_=ps[:])
                    else:
                        nc.vector.tensor_copy(out=dst, in_=ps[:])
                store_eng.dma_start(out=odev[:, g0 * BL:g1 * BL],
                                    in_=obuf[:, :(g1 - g0) * BL])
    nc.compile()
    return nc


# -------------------------------------------------------------- entry point --
def kernel(Y, F, G, H, Q, R, x0, P0):
    import ml_dtypes

    from concourse import bass_utils

    io_dtype = np.float32 if F32_MODE else ml_dtypes.bfloat16
    uniq, band, Ps = _build_operator(F, G, H, Q, R, P0)
    n_uniq = len(uniq)
    wpack = np.ascontiguousarray(
        uniq.transpose(1, 0, 2)).reshape(128, n_uniq * 128).astype(io_dtype)

    nc = _build_bass(n_uniq, band, ych=[1, 2, 4, 8] + [12] * 4, gm=4,
                     ob_bufs=6, ps_bufs=7,
                     io_bf16=not F32_MODE, out_bf16=not F32_MODE)
    ycores = _pack_y(Y, x0, io_dtype)
    in_maps = [{"yin": yc, "win": wpack} for yc in ycores]

    trace = bool(os.environ.get("KF_TRACE"))
    res = bass_utils.run_bass_kernel_spmd(
        nc, in_maps, core_ids=list(range(NCORES)), trace=trace)
    LAST_RESULTS["bass"] = res
    LAST_RESULTS["nc"] = nc
    LAST_RESULTS["in_maps"] = in_maps

    traj = _unpack_out([r["out"] for r in res.results])
    return traj, Ps
